# revision 5
# baseline (speedup 1.0000x reference)
"""Lovasz-Softmax loss on Trainium2 (Bass/Tile), 8-core data-parallel over batch.

Math: for each (batch, class c>=1) the Lovasz term equals
    term = sum_{fg pixels i} Phi(e_i) + 1 - G * Phi(1)
where e_i = 1 - p_c(i), G = #fg pixels, Phi(x) = int_0^x dt / (G + b(t)),
and b(t) = #background-valid pixels with p_c > t.  (Derived from the
sorted-cumsum definition via Abel summation; exact.)

Device work per core (1 batch of 262144 pixels x 21 classes):
  softmax -> p256 = 256*p_c; poison fg/invalid elements negative;
  staircase bits S_a = [p256 >= 32a], T_b = [fmod(p256,32) >= 4b]  (a,b in 0..7)
  -> PE matmuls accumulate Csuf[c,a,b] = sum_pix S_a*T_b  (M=64-bin 2D suffix
  histogram per class) plus fg ramp sums R[c,j] = sum_fg clamp(e*16/256 - j, 0, 1).
Host tail (tiny, O(20*64) per batch): 2D-diff -> b(t) at 64 edges -> trapz Phi
-> term per (b,c); include/count logic; final scalar.
"""

import numpy as np
from contextlib import ExitStack

import concourse.bass as bass
import concourse.tile as tile
from concourse import bacc, mybir
from concourse.bass_utils import run_bass_kernel_spmd

F32 = mybir.dt.float32
BF16 = mybir.dt.bfloat16
ALU = mybir.AluOpType
ACTF = mybir.ActivationFunctionType
AXL = mybir.AxisListType

P = 128
C = 21
NCLS = 20
MHI, MLO, MFG = 8, 8, 16
LVL = 32.0   # hi staircase step in p256 units
LOW = 4.0    # lo staircase step
POIS = 2000.0
N_CORES = 8


def _ap(base, extra_off, dims):
    """Custom AP on a tile/dram AP: keep partition dim, replace free dims."""
    return bass.AP(tensor=base.tensor, offset=base.offset + extra_off,
                   ap=[list(base.ap[0])] + [list(d) for d in dims])


def build(ncols=2048, T=64):
    assert ncols % T == 0
    NT = ncols // T
    nc = bacc.Bacc("TRN2", target_bir_lowering=False, debug=False,
                   enable_asserts=False, num_devices=N_CORES)
    lg_d = nc.dram_tensor("logits", [P, ncols, C], F32, kind="ExternalInput")
    lab_d = nc.dram_tensor("labels", [P, ncols], F32, kind="ExternalInput")
    cst_d = nc.dram_tensor("consts", [P, 36], F32, kind="ExternalInput")
    o1_d = nc.dram_tensor("out1", [128, 128], F32, kind="ExternalOutput")
    o2_d = nc.dram_tensor("out2a", [32, 32], F32, kind="ExternalOutput")
    o3_d = nc.dram_tensor("out2b", [NCLS, MFG], F32, kind="ExternalOutput")

    with tile.TileContext(nc) as tc, ExitStack() as ctx:
        singles = ctx.enter_context(tc.tile_pool(name="singles", bufs=1))
        pool = ctx.enter_context(tc.tile_pool(name="work", bufs=2))
        psum = ctx.enter_context(
            tc.tile_pool(name="psum", bufs=1, space=bass.MemorySpace.PSUM))

        labs = singles.tile([P, ncols], F32)
        nc.sync.dma_start(labs[:], lab_d.ap())
        cst = singles.tile([P, 36], F32)
        nc.sync.dma_start(cst[:], cst_d.ap())

        ps1 = psum.tile([128, 128], F32)   # 16cls x (8lvl) rows, 16cls x (8lo) cols
        ps2 = psum.tile([32, 32], F32)     # classes 16..19
        ps3 = psum.tile([NCLS, MFG], F32)  # fg ramps

        lg_ap = lg_d.ap()
        labs_ap = labs[:]
        cst_ap = cst[:]

        for it in range(NT):
            t0 = it * T
            lgt = pool.tile([P, T, C], F32, tag="lg")
            nc.sync.dma_start(
                lgt[:], _ap(lg_ap, t0 * C, [[C, T], [1, C]]))
            ez = pool.tile([P, T, C], F32, tag="ez")
            nc.scalar.activation(ez[:], lgt[:], ACTF.Exp)
            s = pool.tile([P, T], F32, tag="s")
            nc.vector.tensor_reduce(s[:], ez[:], axis=AXL.X, op=ALU.add)
            rc = pool.tile([P, T], F32, tag="rc")
            nc.vector.reciprocal(rc[:], s[:])
            rc256 = pool.tile([P, T], F32, tag="rc256")
            nc.vector.tensor_scalar(rc256[:], rc[:], 256.0, None, ALU.mult)
            poisA = pool.tile([P, T], F32, tag="poisA")
            nc.vector.tensor_scalar(
                poisA[:], _ap(labs_ap, t0, [[1, T]]), 0.0, POIS,
                ALU.is_equal, ALU.mult)

            # fg one-hot over classes 1..20 (f32 + bf16 copy for PE)
            fgm = pool.tile([P, T, NCLS], F32, tag="fgm")
            nc.vector.tensor_tensor(
                fgm[:],
                _ap(labs_ap, t0, [[1, T], [0, NCLS]]),
                _ap(cst_ap, 0, [[0, T], [1, NCLS]]),
                ALU.is_equal)
            fgmh = pool.tile([P, T, NCLS], BF16, tag="fgmh")
            nc.vector.tensor_copy(fgmh[:], fgm[:])

            # pois = fgm*2000 + poisA (broadcast over class)
            pois = pool.tile([P, T, NCLS], F32, tag="pois")
            nc.vector.scalar_tensor_tensor(
                pois[:], fgm[:], POIS,
                _ap(poisA[:], 0, [[1, T], [0, NCLS]]),
                op0=ALU.mult, op1=ALU.add)

            # p256 for classes 1..20, then poisoned pp
            p1 = pool.tile([P, T, NCLS], F32, tag="p1")
            nc.vector.tensor_tensor(
                p1[:],
                _ap(ez[:], 1, [[C, T], [1, NCLS]]),
                _ap(rc256[:], 0, [[1, T], [0, NCLS]]),
                ALU.mult)
            pp = pool.tile([P, T, NCLS], F32, tag="pp")
            nc.vector.tensor_tensor(pp[:], p1[:], pois[:], ALU.subtract)

            # fg pixel value: qfg_raw = sum_c fgm*pp  (= p256_at_label - 2000 on valid)
            tmp = pool.tile([P, T, NCLS], F32, tag="tmp")
            nc.vector.tensor_tensor(tmp[:], fgm[:], pp[:], ALU.mult)
            qfg = pool.tile([P, T], F32, tag="qfg")
            nc.vector.tensor_reduce(qfg[:], tmp[:], axis=AXL.X, op=ALU.add)
            # x16 = e256/16 = (256 - (qfg_raw+2000))/16 = -qfg_raw/16 - 109
            x16 = pool.tile([P, T], F32, tag="x16")
            nc.vector.tensor_scalar(
                x16[:], qfg[:], -1.0 / 16.0, -109.0, ALU.mult, ALU.add)
            u = pool.tile([P, T, MFG], F32, tag="u")
            nc.vector.tensor_tensor(
                u[:],
                _ap(x16[:], 0, [[1, T], [0, MFG]]),
                _ap(cst_ap, 20, [[0, T], [1, MFG]]),
                ALU.subtract)
            rmp = pool.tile([P, T, MFG], BF16, tag="rmp")
            nc.vector.tensor_scalar(rmp[:], u[:], 1.0, 0.0, ALU.min, ALU.max)

            # staircases, split into class groups 0..15 / 16..19 so each
            # per-chunk matmul operand is a single contiguous run
            # (walrus: matmul APs may have only one free dimension).
            # Layout [P, T, lvl, cls] -> per (pixel, t) a flat lvl-major block.
            # qi = floor(pp) as int16 (f32->i16 conversion is round-nearest-even,
            # so subtract 0.5 first); lo residue via bitwise_and.
            I16 = mybir.dt.int16
            qi = pool.tile([P, T, NCLS], I16, tag="qi")
            nc.vector.tensor_scalar(qi[:], pp[:], -0.5, None, ALU.add)
            mi = pool.tile([P, T, NCLS], I16, tag="mi")
            nc.vector.tensor_scalar(mi[:], qi[:], 31, None, ALU.bitwise_and)

            St1 = pool.tile([P, T, MHI, 16], BF16, tag="St1")
            St2 = pool.tile([P, T, MHI, 4], BF16, tag="St2")
            qi_ap = qi[:]
            for a in range(MHI):
                nc.vector.tensor_scalar(
                    _ap(St1[:], a * 16, [[MHI * 16, T], [1, 16]]),
                    _ap(qi_ap, 0, [[NCLS, T], [1, 16]]),
                    int(LVL) * a, None, ALU.is_ge)
                nc.vector.tensor_scalar(
                    _ap(St2[:], a * 4, [[MHI * 4, T], [1, 4]]),
                    _ap(qi_ap, 16, [[NCLS, T], [1, 4]]),
                    int(LVL) * a, None, ALU.is_ge)
            Tt1 = pool.tile([P, T, MLO, 16], BF16, tag="Tt1")
            Tt2 = pool.tile([P, T, MLO, 4], BF16, tag="Tt2")
            mi_ap = mi[:]
            for b in range(MLO):
                nc.vector.tensor_scalar(
                    _ap(Tt1[:], b * 16, [[MLO * 16, T], [1, 16]]),
                    _ap(mi_ap, 0, [[NCLS, T], [1, 16]]),
                    int(LOW) * b, None, ALU.is_ge)
                nc.vector.tensor_scalar(
                    _ap(Tt2[:], b * 4, [[MLO * 4, T], [1, 4]]),
                    _ap(mi_ap, 16, [[NCLS, T], [1, 4]]),
                    int(LOW) * b, None, ALU.is_ge)

            # per-chunk matmuls, accumulating in PSUM
            S1_ap = St1[:]
            S2_ap = St2[:]
            T1_ap = Tt1[:]
            T2_ap = Tt2[:]
            F_ap = fgmh[:]
            R_ap = rmp[:]
            for tt in range(T):
                first = (it == 0 and tt == 0)
                last = (it == NT - 1 and tt == T - 1)
                nc.tensor.matmul(
                    ps1[:],
                    _ap(S1_ap, tt * MHI * 16, [[1, MHI * 16]]),
                    _ap(T1_ap, tt * MLO * 16, [[1, MLO * 16]]),
                    start=first, stop=last)
                nc.tensor.matmul(
                    ps2[:],
                    _ap(S2_ap, tt * MHI * 4, [[1, MHI * 4]]),
                    _ap(T2_ap, tt * MLO * 4, [[1, MLO * 4]]),
                    start=first, stop=last)
                nc.tensor.matmul(
                    ps3[:],
                    _ap(F_ap, tt * NCLS, [[1, NCLS]]),
                    _ap(R_ap, tt * MFG, [[1, MFG]]),
                    start=first, stop=last)

        o1s = singles.tile([128, 128], F32)
        nc.vector.tensor_copy(o1s[:], ps1[:])
        nc.sync.dma_start(o1_d.ap(), o1s[:])
        o2s = singles.tile([32, 32], F32)
        nc.vector.tensor_copy(o2s[:], ps2[:])
        nc.sync.dma_start(o2_d.ap(), o2s[:])
        o3s = singles.tile([NCLS, MFG], F32)
        nc.vector.tensor_copy(o3s[:], ps3[:])
        nc.sync.dma_start(o3_d.ap(), o3s[:])

    nc.compile()
    return nc


def make_consts():
    row = np.concatenate([np.arange(1, 21), np.arange(16)]).astype(np.float32)
    return np.ascontiguousarray(np.broadcast_to(row, (P, 36)))


def host_tail(out1, out2a, out2b, label_counts):
    """Per-batch: decode device accumulators -> 20 Lovasz terms (float64)."""
    M = MHI * MLO
    terms = np.zeros(NCLS)
    for ci in range(NCLS):
        if ci < 16:
            cs = out1[ci::16, ci::16]          # [MHI, MLO], rows lvl-major
        else:
            k = ci - 16
            cs = out2a[k::4, k::4]
        cs = cs.astype(np.float64)              # Csuf[a, b] suffix-suffix counts
        h = cs.copy()
        h[:-1, :] -= cs[1:, :]
        h[:, :-1] -= h[:, 1:]
        hflat = h.reshape(-1)
        bcnt = np.concatenate([np.cumsum(hflat[::-1])[::-1], [0.0]])  # b at M+1 edges
        g = float(label_counts[ci + 1]) if ci + 1 < len(label_counts) else 0.0
        r = 1.0 / np.maximum(g + bcnt, 1.0)
        dt = 1.0 / M
        phi = np.concatenate([[0.0], np.cumsum((r[:-1] + r[1:]) * 0.5 * dt)])
        stride = M // MFG
        dphi = np.diff(phi[::stride])
        fg_term = float((dphi * out2b[ci].astype(np.float64)).sum())
        terms[ci] = fg_term + 1.0 - g * phi[-1]
    return terms


_NC_CACHE = {}


def _get_nc():
    if "nc" not in _NC_CACHE:
        _NC_CACHE["nc"] = build()
    return _NC_CACHE["nc"]


def kernel(logits, labels):
    B, N, Cin = logits.shape
    assert (B, Cin) == (N_CORES, C) and N % P == 0
    ncols = N // P
    logits32 = np.ascontiguousarray(logits, dtype=np.float32).reshape(B, P, ncols, C)
    labf = np.ascontiguousarray(labels.astype(np.float32).reshape(B, P, ncols))
    consts = make_consts()
    nc = _get_nc()
    in_maps = [
        {"logits": logits32[b], "labels": labf[b], "consts": consts}
        for b in range(B)
    ]
    res = run_bass_kernel_spmd(nc, in_maps, core_ids=list(range(N_CORES)))
    _NC_CACHE["last_results"] = res

    lab_int = labels.astype(np.int64)
    total = 0.0
    n_included = 0
    for b in range(B):
        out = res.results[b]
        counts = np.bincount(lab_int[b], minlength=C)
        terms = host_tail(out["out1"], out["out2a"], out["out2b"], counts)
        valid = int(N - counts[0])
        if valid >= 2:
            total += terms.sum()
            n_included += 1
    count = max(n_included * (C - 1), 1)
    return np.float32(total / count)
